# revision 2
# baseline (speedup 1.0000x reference)
"""Trainium2 Bass kernel for nn_GSCAN_model (gnn_message_passing).

Reference computation (per cell of a [B, 32, 32, 17] grid):
    emb    = concat(x[0:4] @ W_size, x[4:8] @ W_shape,
                    x[8:12] @ W_rgb, x[12:17] @ W_agent)     # [64]
    mask   = sum(x) > 0
    out    = mask ? emb : [x, zeros(47)]                     # [64]

Kernel formulation: fold the mask into the matmul.
    out = (x*m) @ (Wblk - P17)  +  pad(x)
where Wblk is the 17x64 block-diagonal assembly of the four small
weights and P17 embeds the 17 raw channels.  For masked-off cells the
matmul contribution is exactly zero, so adding raw x on the first 17
channels reproduces the padded passthrough bit-exactly.

This is memory-bound: 68 B in + 256 B out per cell.  Layout: macro
tiles of 128 partitions x 128 cells; per partition the input run is
8704 B and the output run 32 KiB contiguous, so both DMA directions use
large descriptors.  Loads issue on the SP HWDGE ring, stores on the ACT
ring.  The tensor path runs in bf16 (1 PE cycle/row vs 4 for fp32):
per macro, 19 PE transposes batch 7 cell-slots each ([128,119]->[119,
128]) and 19 bf16 matmuls against a block-diagonal weight Wd [119,448]
land cells back on partitions.  The raw-x passthrough is fused into the
PSUM->SBUF drain (17-ch add + 47-ch copy per group).

Data parallel over 8 NeuronCores: batch dim 2048 -> 256 per core.
"""

import numpy as np
import ml_dtypes

B, H, W, C_IN = 2048, 32, 32, 17
EMB = 64
N_CORES = 8
P = 128                      # partitions
C_SLOTS = 128                # cells per partition per macro tile
CELLS_PER_CORE = (B // N_CORES) * H * W          # 262144
MACROS = CELLS_PER_CORE // (P * C_SLOTS)         # 16
# groups of cell-slots per macro: 18 groups of 7 slots + 1 group of 2
GROUPS = [(7 * i, 7) for i in range(18)] + [(126, 2)]
KW = 7 * C_IN                # 119 rows: largest weight-block group
NW = 7 * EMB                 # 448 cols

_CACHE = {}


def _build_program(n_macros):
    import concourse.bacc as bacc
    import concourse.mybir as mybir
    from concourse.tile import TileContext

    f32 = mybir.dt.float32
    bf16 = mybir.dt.bfloat16
    nc = bacc.Bacc("TRN2", target_bir_lowering=False, debug=False,
                   num_devices=N_CORES)

    cells = n_macros * P * C_SLOTS
    x = nc.dram_tensor("x", [cells, C_IN], f32, kind="ExternalInput")
    wd = nc.dram_tensor("wd", [KW, NW], bf16, kind="ExternalInput")
    ident = nc.dram_tensor("ident", [P, P], bf16, kind="ExternalInput")
    y = nc.dram_tensor("y", [cells, EMB], f32, kind="ExternalOutput")

    xr = x.ap().rearrange("(m p c) k -> m p (c k)", p=P, c=C_SLOTS)
    yr = y.ap().rearrange("(m p c) n -> m p (c n)", p=P, c=C_SLOTS)

    # transpose destinations: quads of 4 groups share one 2 KB PSUM bank
    QUADS = [GROUPS[q * 4:(q + 1) * 4] for q in range(5)]

    with TileContext(nc) as tc:
        with (
            tc.tile_pool(name="const", bufs=1) as constp,
            tc.tile_pool(name="xin", bufs=3) as xin_pool,
            tc.tile_pool(name="sum", bufs=2) as s_pool,
            tc.tile_pool(name="xm", bufs=2) as xm_pool,
            tc.tile_pool(name="xat", bufs=2) as xat_pool,
            tc.tile_pool(name="outp", bufs=3) as out_pool,
            tc.tile_pool(name="pst", bufs=2, space="PSUM") as pst_pool,
            tc.tile_pool(name="pso", bufs=6, space="PSUM") as pso_pool,
        ):
            wd_t = constp.tile([KW, NW], bf16)
            nc.sync.dma_start(out=wd_t, in_=wd.ap())
            id_t = constp.tile([P, P], bf16)
            nc.sync.dma_start(out=id_t, in_=ident.ap())

            for mi in range(n_macros):
                xt = xin_pool.tile([P, C_SLOTS * C_IN], f32)
                nc.sync.dma_start(out=xt, in_=xr[mi])
                xt3 = xt.rearrange("p (c k) -> p c k", k=C_IN)

                # mask fold on GPSIMD (SBUF-only engine, otherwise idle):
                # xm = (sum_k(x) > 0) * x, cast to bf16 for the PE.
                s_t = s_pool.tile([P, C_SLOTS], f32)
                nc.gpsimd.tensor_reduce(out=s_t, in_=xt3,
                                        axis=mybir.AxisListType.X,
                                        op=mybir.AluOpType.add)
                xm = xm_pool.tile([P, C_SLOTS * C_IN], bf16)
                xm3 = xm.rearrange("p (c k) -> p c k", k=C_IN)
                s_b = s_t.unsqueeze(2).broadcast_to((P, C_SLOTS, C_IN))
                nc.gpsimd.scalar_tensor_tensor(
                    out=xm3, in0=s_b, scalar=0.0, in1=xt3,
                    op0=mybir.AluOpType.is_gt, op1=mybir.AluOpType.mult)

                # Phase 1: PE transposes cell-slot groups to channel-major.
                tps = []
                for quad in QUADS:
                    tp = pst_pool.tile([P, 4 * P], f32, tag="tp")
                    for j, (c0, ns) in enumerate(quad):
                        k = ns * C_IN
                        nc.tensor.transpose(
                            out=tp[0:k, j * P:(j + 1) * P],
                            in_=xm[:, c0 * C_IN:(c0 + ns) * C_IN],
                            identity=id_t)
                    tps.append(tp)

                # drain PSUM -> SBUF (cast to bf16) so matmul can use lhsT
                xat = xat_pool.tile([P, len(GROUPS) * P], bf16)
                for gi, (c0, ns) in enumerate(GROUPS):
                    k = ns * C_IN
                    src = tps[gi // 4][0:k, (gi % 4) * P:(gi % 4 + 1) * P]
                    nc.scalar.copy(out=xat[0:k, gi * P:(gi + 1) * P],
                                   in_=src)

                # Phase 2: one matmul per group; output lands cells-on-
                # partitions.  Drain fuses the raw-x passthrough: 17-ch
                # tensor add on DVE, 47-ch copy split DVE/ACT.
                out_t = out_pool.tile([P, C_SLOTS * EMB], f32)
                out3 = out_t.rearrange("p (c n) -> p c n", n=EMB)
                for gi, (c0, ns) in enumerate(GROUPS):
                    k = ns * C_IN
                    n = ns * EMB
                    po = pso_pool.tile([P, NW], f32, tag="po")
                    nc.tensor.matmul(out=po[:, 0:n],
                                     lhsT=xat[0:k, gi * P:(gi + 1) * P],
                                     rhs=wd_t[0:k, 0:n],
                                     start=True, stop=True)
                    po3 = po.rearrange("p (c n) -> p c n", n=EMB)
                    nc.vector.tensor_tensor(
                        out=out3[:, c0:c0 + ns, 0:C_IN],
                        in0=po3[:, 0:ns, 0:C_IN],
                        in1=xt3[:, c0:c0 + ns, :],
                        op=mybir.AluOpType.add)
                    dst = out3[:, c0:c0 + ns, C_IN:EMB]
                    src = po3[:, 0:ns, C_IN:EMB]
                    if gi % 2 == 0:
                        nc.vector.tensor_copy(out=dst, in_=src)
                    else:
                        nc.scalar.copy(out=dst, in_=src)

                # store on the ACT HWDGE ring; loads use the SP ring
                nc.scalar.dma_start(out=yr[mi], in_=out_t)
    nc.compile()
    return nc


def _host_weights(W_size, W_shape, W_rgb, W_agent):
    """Wd [119, 448] bf16: 7 diagonal blocks of (Wblk - P17) [17, 64].

    Per slot the kernel feeds X*m; (X*m) @ (Wblk - P17) + X equals
    where(m, emb, pad(X)) -- the +X on channels 0:17 is applied during
    the PSUM drain.
    """
    wblk = np.zeros((C_IN, EMB), np.float32)
    wblk[0:4, 0:16] = W_size
    wblk[4:8, 16:32] = W_shape
    wblk[8:12, 32:48] = W_rgb
    wblk[12:17, 48:64] = W_agent
    pad = np.zeros((C_IN, EMB), np.float32)
    pad[np.arange(C_IN), np.arange(C_IN)] = 1.0
    w17 = wblk - pad                                 # [17, 64]
    wd = np.zeros((KW, NW), np.float32)
    for i in range(7):
        wd[i * C_IN:(i + 1) * C_IN, i * EMB:(i + 1) * EMB] = w17
    return wd.astype(ml_dtypes.bfloat16)


def _in_maps(situation, W_size, W_shape, W_rgb, W_agent):
    wd = _host_weights(np.asarray(W_size, np.float32),
                       np.asarray(W_shape, np.float32),
                       np.asarray(W_rgb, np.float32),
                       np.asarray(W_agent, np.float32))
    ident = np.eye(P, dtype=ml_dtypes.bfloat16)
    sit = np.ascontiguousarray(np.asarray(situation), dtype=np.float32)
    bpc = B // N_CORES
    in_maps = []
    for i in range(N_CORES):
        shard = sit[i * bpc:(i + 1) * bpc].reshape(CELLS_PER_CORE, C_IN)
        in_maps.append({"x": np.ascontiguousarray(shard),
                        "wd": wd, "ident": ident})
    return in_maps


def kernel(situation, W_size, W_shape, W_rgb, W_agent):
    from concourse.bass_utils import run_bass_kernel_spmd

    key = "prog"
    if key not in _CACHE:
        _CACHE[key] = _build_program(MACROS)
    nc = _CACHE[key]

    in_maps = _in_maps(situation, W_size, W_shape, W_rgb, W_agent)
    res = run_bass_kernel_spmd(nc, in_maps, core_ids=list(range(N_CORES)))
    bpc = B // N_CORES
    out = np.empty((B, H, W, EMB), np.float32)
    for i in range(N_CORES):
        out[i * bpc:(i + 1) * bpc] = res.results[i]["y"].reshape(
            bpc, H, W, EMB)
    return out
